# revision 41
# baseline (speedup 1.0000x reference)
"""Trainium2 Bass kernel for an encoder block (dense transformer).

Problem: x[8,1024,768]; fused qkvr projection (innermost-4 interleave),
softmax(unscaled logits)/sqrt(768), r-gate, proj + residual + postLN,
exact-gelu FFN (768->3072->768) + residual + postLN.

Strategy: data-parallel over batch - one batch element per NeuronCore,
no collectives. The qkvr/v/proj/ff1 matmuls run in fp8(e4m3) DoubleRow
mode (2x PE rate); ff2 stays bf16 (its quantization error lands
directly on the output and would blow the error budget). Energy and
att@v run bf16. All fp8 operands carry power-of-2 scales folded into
downstream ops for free: exp(scale*energy), gelu(scale*h), and
scalar_tensor_tensor descale-on-evacuate. x arrives pre-transposed and
pre-quantized from the host; softmax uses a ones-column appended to v
to get per-query exp-sums from the same matmul, normalization applied
after att@v. Heads are software-pipelined (qkr of head h+1 issued
between exp and att@v of head h) so the tensor queue never stalls on
the scalar-engine exps.
"""

import sys

if "/opt/trn_rl_repo" not in sys.path:
    sys.path.insert(0, "/opt/trn_rl_repo")

from contextlib import ExitStack

import numpy as np
import ml_dtypes

import concourse.bass as bass
import concourse.mybir as mybir
import concourse.tile as tile
from concourse import bacc
from concourse.bass_utils import run_bass_kernel_spmd
from concourse.masks import make_identity

F32 = mybir.dt.float32
F32R = mybir.dt.float32r
BF16 = mybir.dt.bfloat16
F8 = mybir.dt.float8e4
F8E5 = mybir.dt.float8e5
AF = mybir.ActivationFunctionType
ALU = mybir.AluOpType
DR = mybir.MatmulPerfMode.DoubleRow

N_CORES = 8
B, N, E = 8, 1024, 768
H, D = 8, 96          # heads, head dim
C = 4 * E             # ffn hidden 3072
NQT = N // 128        # 8 seq tiles
NEC = E // 128        # 6 embedding chunks
NPR = NEC // 2        # 3 e-chunk pairs (DoubleRow)
NCT = C // 128        # 24 ffn chunks
LN_EPS = 1e-5
ESPL = [(0, 512), (512, 256)]  # bank-aligned 768 split for psum outputs

# power-of-2 fp8 scales; all descales fold into existing ops
SX = 16.0            # x and x1 activations (2^4)
SW = 512.0           # wq/wk/wr/wproj/wff1 (2^9)
SWV = 8192.0         # wv (1/sqrt(E) pre-folded, needs more lift) (2^13)
V8_DESCALE = 1.0 / 2.0**11   # v psum (2^17) -> e4m3 v_aug at v_true*2^6
C1 = 128.0           # ones-column in v_aug (must fit e4m3)
SUMS_SCALE = 16.0    # sums staging lift so og8 = og_true * 2^8
EXP_OFF = -12.0      # global exp offset: softmax-invariant, fits e4m3 window
EXP_SCALE = 1.0 / (SX * SW) ** 2       # 2^-26
PROJ_DESCALE = 1.0 / (2.0**8 * SW)     # 2^-17
FF1_DESCALE = 1.0 / (SX * SW)          # 2^-13
FF2_DESCALE = 1.0 / SWV                # 2^-13 (ff2_fp8 variant only)


def _bcast_dma(nc, out_ap, row_ap):
    """Replicate a DRAM row across partitions (partition-step-0 source)."""
    src = bass.AP(
        tensor=row_ap.tensor,
        offset=row_ap.offset,
        ap=[[0, out_ap.shape[0]], list(row_ap.ap[-1])],
    )
    nc.gpsimd.dma_start(out=out_ap, in_=src)


def _ln_post(nc, pool, t1, scr, dst, g_bc, b_bc, eps_t, identity_ln, on_scalar):
    """LN(t1) -> dst. scr[:,0:1] must already hold sum(t1) (producer's
    accum_out). Square/Identity live in every act table (no table loads);
    stats come from row sums instead of bn_stats to keep vector light.
    on_scalar picks the engine for the two [128,E] passes (square, apply)
    so callers can balance scalar vs vector load per phase."""
    s1, s2, m, msq, v, sq, r, nb = (scr[:, i : i + 1] for i in range(8))
    junk = pool.tile([128, E], F32, tag="lnjunk", name="lnjunk", bufs=1)
    if on_scalar:
        nc.scalar.activation(out=junk[:], in_=t1, func=AF.Square, accum_out=s2)
    else:
        nc.vector.scalar_tensor_tensor(
            out=junk[:], in0=t1, scalar=1.0, in1=t1,
            op0=ALU.mult, op1=ALU.mult, accum_out=s2,
        )
    nc.vector.tensor_scalar(out=m, in0=s1, scalar1=1.0 / E, scalar2=None, op0=ALU.mult)
    nc.vector.tensor_tensor(out=msq, in0=m, in1=m, op=ALU.mult)
    nc.vector.scalar_tensor_tensor(
        out=v, in0=s2, scalar=1.0 / E, in1=msq, op0=ALU.mult, op1=ALU.subtract,
    )
    nc.scalar.activation(out=sq, in_=v, func=AF.Sqrt, bias=eps_t[:], scale=1.0)
    nc.vector.reciprocal(r, sq)
    if on_scalar:
        nc.vector.tensor_scalar(out=nb, in0=m, scalar1=r, scalar2=-1.0, op0=ALU.mult, op1=ALU.mult)
    t2a = dst if identity_ln else pool.tile([128, E], F32, tag="lnt2", name="lnt2")[:]
    if on_scalar:
        nc.scalar.activation(out=t2a, in_=t1, func=AF.Identity, bias=nb, scale=r)
    else:
        nc.vector.tensor_scalar(
            out=t2a, in0=t1, scalar1=m, scalar2=r, op0=ALU.subtract, op1=ALU.mult,
        )
    if not identity_ln:
        nc.vector.tensor_tensor(out=t2a, in0=t2a, in1=g_bc, op=ALU.mult)
        nc.vector.tensor_tensor(out=dst, in0=t2a, in1=b_bc, op=ALU.add)


def _build(identity_ln=False, zero_bias=False, ff2_fp8=False):
    nc = bacc.Bacc(num_devices=N_CORES)

    x_d = nc.declare_dram_parameter("x", [N, E], F32, isOutput=False)
    xt8_d = nc.declare_dram_parameter("xt8", [128, NEC, N], F8, isOutput=False)
    wq8_d = nc.declare_dram_parameter("wq8", [128, H, NPR, 2, D], F8, isOutput=False)
    wk8_d = nc.declare_dram_parameter("wk8", [128, H, NPR, 2, D], F8, isOutput=False)
    wr8_d = nc.declare_dram_parameter("wr8", [128, H, NPR, 2, D], F8, isOutput=False)
    wv8_d = nc.declare_dram_parameter("wv8", [128, NPR, 2, E], F8, isOutput=False)
    wp8_d = nc.declare_dram_parameter("wp8", [D, H // 2, 2, E], F8, isOutput=False)
    w18_d = nc.declare_dram_parameter("w18", [128, NCT, NPR, 2, 128], F8, isOutput=False)
    bqkr_d = nc.declare_dram_parameter("bqkr", [D, 3, H], F32, isOutput=False)
    bv_d = nc.declare_dram_parameter("bv", [1, E], F32, isOutput=False)
    bproj_d = nc.declare_dram_parameter("bproj", [1, E], F32, isOutput=False)
    ln1g_d = nc.declare_dram_parameter("ln1g", [1, E], F32, isOutput=False)
    ln1b_d = nc.declare_dram_parameter("ln1b", [1, E], F32, isOutput=False)
    bff1_d = nc.declare_dram_parameter("bff1", [128, NCT], F32, isOutput=False)
    w2_dt = F8 if ff2_fp8 else BF16
    w2_d = nc.declare_dram_parameter("w2", [128, NCT, E], w2_dt, isOutput=False)
    bff2_d = nc.declare_dram_parameter("bff2", [1, E], F32, isOutput=False)
    ln2g_d = nc.declare_dram_parameter("ln2g", [1, E], F32, isOutput=False)
    ln2b_d = nc.declare_dram_parameter("ln2b", [1, E], F32, isOutput=False)
    y_d = nc.declare_dram_parameter("y", [N, E], F32, isOutput=True)

    with tile.TileContext(nc) as tc, ExitStack() as ctx:
        # ---- whole-kernel pools ----
        persist = ctx.enter_context(tc.tile_pool(name="persist", bufs=1))
        wts = ctx.enter_context(tc.tile_pool(name="wts", bufs=1))
        act = ctx.enter_context(tc.tile_pool(name="act", bufs=1))

        ident = persist.tile([128, 128], F32)
        make_identity(nc, ident[:])
        eps_t = persist.tile([128, 1], F32)
        nc.vector.memset(eps_t[:], LN_EPS)
        expoff_t = persist.tile([128, 1], F32)
        nc.vector.memset(expoff_t[:], EXP_OFF)

        # ---- weight/x DMAs, transfer order = issue order: qkr(h0) operands
        # first so the tensor engine starts ~3us in ----
        xT8 = act.tile([128, NEC, N], F8, name="xT8")
        nc.sync.dma_start(out=xT8[:, :, 0:512], in_=xt8_d[:, :, 0:512])
        wq_sb = wts.tile([128, H, NPR, 2, D], F8, name="wq_sb")
        wk_sb = wts.tile([128, H, NPR, 2, D], F8, name="wk_sb")
        wr_sb = wts.tile([128, H, NPR, 2, D], F8, name="wr_sb")
        wv_sb = wts.tile([128, NPR, 2, E], F8, name="wv_sb")
        nc.sync.dma_start(out=wq_sb[:], in_=wq8_d[:])
        nc.sync.dma_start(out=wv_sb[:], in_=wv8_d[:])
        nc.sync.dma_start(out=wk_sb[:], in_=wk8_d[:])
        nc.sync.dma_start(out=wr_sb[:], in_=wr8_d[:])
        nc.sync.dma_start(out=xT8[:, :, 512:1024], in_=xt8_d[:, :, 512:1024])
        wp_sb = wts.tile([D, H // 2, 2, E], F8, name="wp_sb")
        nc.sync.dma_start(out=wp_sb[:], in_=wp8_d[:])
        # bulk weights last on the same queue: they transfer only after the
        # attention-critical items above, but well before ff1 needs them
        w1_sb = wts.tile([128, NCT, NPR, 2, 128], F8, name="w1_sb")
        nc.sync.dma_start(out=w1_sb[:], in_=w18_d[:])
        bff1_t = wts.tile([128, NCT], F32, name="bff1_t")
        nc.sync.dma_start(out=bff1_t[:], in_=bff1_d[:])
        bqkr_t = persist.tile([D, 3, H], F32)
        nc.sync.dma_start(out=bqkr_t[:], in_=bqkr_d[:])

        # warm-up matmuls (PE clock ramp)
        warm_t = persist.tile([128, 128], BF16)
        nc.vector.memset(warm_t[:], 0.0)
        with tc.tile_pool(name="warm_ps", bufs=2, space="PSUM") as warm_ps:
            for _ in range(12):
                wp_ = warm_ps.tile([128, 128], F32, tag="wp_", name="wp_")
                nc.tensor.matmul(wp_[:], warm_t[:], warm_t[:], start=True, stop=True)
                nc.tensor.matmul(wp_[:], warm_t[:], warm_t[:], start=True, stop=True)

        # ---- phases 1+2: v projection and attention share the qkr pools so
        # qkr(head 0) can be emitted before the v loop (fills the DMA wait) ----
        # last dim padded 97->128: dual-fp8 LDWEIGHTS requires full 32-aligned
        # chunk geometry; pad columns stay zero and emit into unused psum rows
        DP = 128
        v_aug = act.tile([128, H, NQT, DP], F8, name="v_aug")
        nc.vector.memset(v_aug[:, :, :, D : D + 1], C1)
        nc.vector.memset(v_aug[:, :, :, D + 1 : DP], 0.0)
        og8 = act.tile([D, H, N], F8, name="og8")  # gated out, scale 2^8
        with (
            tc.tile_pool(name="qkr", bufs=2) as qkr_pool,
            tc.tile_pool(name="expET", bufs=2) as exp_pool,
            tc.tile_pool(name="att_tmp", bufs=2) as tmp_pool,
            tc.tile_pool(name="qkr_ps", bufs=2, space="PSUM") as qkr_ps,
        ):
            def emit_qkr(h):
                dst = qkr_pool.tile([D, 3, N], BF16, tag="qkrT", name=f"qkrT{h}")
                for qh in range(2):
                    for si, w_sb in enumerate((wq_sb, wk_sb, wr_sb)):
                        qp = qkr_ps.tile([D, 512], F32, tag="qp", name="qp")
                        for p in range(NPR):
                            nc.tensor.matmul(
                                qp[:],
                                w_sb[:, h, p, :, :],
                                xT8[:, 2 * p : 2 * p + 2, qh * 512 : (qh + 1) * 512],
                                start=(p == 0),
                                stop=(p == NPR - 1),
                                perf_mode=DR,
                            )
                        dslc = dst[:, si, qh * 512 : (qh + 1) * 512]
                        if zero_bias:
                            nc.vector.tensor_copy(dslc, qp[:])
                        else:
                            nc.vector.tensor_scalar(
                                out=dslc, in0=qp[:],
                                scalar1=bqkr_t[:, si, h : h + 1], scalar2=None,
                                op0=ALU.add,
                            )
                return dst

            with tc.tile_pool(name="v_ps", bufs=2, space="PSUM") as v_ps:
                qkrT = emit_qkr(0)
                bv_bc = None
                if not zero_bias:
                    bv_bc = tmp_pool.tile([128, E], F32, tag="bv", name="bv_bc", bufs=1)
                    _bcast_dma(nc, bv_bc[:], bv_d[0:1, :])
                for qt in range(NQT):
                    vp = v_ps.tile([128, E], F32, tag="vp", name="vp")
                    for p in range(NPR):
                        for o, w in ESPL:
                            nc.tensor.matmul(
                                vp[:, o : o + w],
                                xT8[:, 2 * p : 2 * p + 2, qt * 128 : (qt + 1) * 128],
                                wv_sb[:, p, :, o : o + w],
                                start=(p == 0),
                                stop=(p == NPR - 1),
                                perf_mode=DR,
                            )
                    vph = vp.rearrange("k (h d) -> k h d", h=H)
                    if zero_bias:
                        nc.vector.tensor_scalar(
                            out=v_aug[:, :, qt, 0:D], in0=vph,
                            scalar1=V8_DESCALE, scalar2=None, op0=ALU.mult,
                        )
                    else:
                        nc.vector.scalar_tensor_tensor(
                            out=v_aug[:, :, qt, 0:D],
                            in0=vph, scalar=V8_DESCALE,
                            in1=bv_bc.rearrange("k (h d) -> k h d", h=H),
                            op0=ALU.mult, op1=ALU.add,
                        )
            with (
                tc.tile_pool(name="eng_ps", bufs=2, space="PSUM") as eng_ps,
                tc.tile_pool(name="att_ps", bufs=2, space="PSUM") as att_ps,
            ):
                for h in range(H):
                    cur = qkrT
                    expET = exp_pool.tile([128, NQT, N], F8, tag="expET", name=f"expET{h}")
                    for qh in range(2):
                        for g in range(4):
                            ep = eng_ps.tile([128, 2, 512], F32, tag="ep", name="ep")
                            for j in range(2):
                                kt = 2 * g + j
                                nc.tensor.matmul(
                                    ep[:, j, :],
                                    cur[:, 1, kt * 128 : (kt + 1) * 128],
                                    cur[:, 0, qh * 512 : (qh + 1) * 512],
                                    start=True,
                                    stop=True,
                                )
                            nc.scalar.activation(
                                out=expET[:, 2 * g : 2 * g + 2, qh * 512 : (qh + 1) * 512],
                                in_=ep[:],
                                func=AF.Exp,
                                scale=EXP_SCALE,
                                bias=expoff_t[:],
                            )
                    if h + 1 < H:
                        qkrT = emit_qkr(h + 1)
                    for qh in range(2):
                        op_ = att_ps.tile([128, 512], F32, tag="op", name="op")
                        for t2 in range(NQT // 2):
                            nc.tensor.matmul(
                                op_[:],
                                v_aug[:, h, 2 * t2 : 2 * t2 + 2, :],
                                expET[:, 2 * t2 : 2 * t2 + 2, qh * 512 : (qh + 1) * 512],
                                start=(t2 == 0),
                                stop=(t2 == NQT // 2 - 1),
                                perf_mode=DR,
                            )
                        sums = tmp_pool.tile([1, 512], F32, tag="sums", name="sums")
                        nc.vector.tensor_scalar(
                            out=sums[:], in0=op_[D : D + 1, :],
                            scalar1=SUMS_SCALE, scalar2=None, op0=ALU.mult,
                        )
                        recip = tmp_pool.tile([1, 512], F32, tag="recip", name="recip")
                        nc.vector.reciprocal_approx_fast(recip[:], sums[:])
                        bc = tmp_pool.tile([D, 512], F32, tag="bc", name="bc")
                        nc.gpsimd.partition_broadcast(bc[:], recip[:])
                        gated = tmp_pool.tile([D, 512], F32, tag="gated", name="gated")
                        nc.vector.tensor_tensor(
                            out=gated[:],
                            in0=op_[0:D, :],
                            in1=cur[:, 2, qh * 512 : (qh + 1) * 512],
                            op=ALU.mult,
                        )
                        # pure-SBUF multiply: offload to the idle gpsimd engine
                        nc.gpsimd.tensor_tensor(
                            out=og8[:, h, qh * 512 : (qh + 1) * 512],
                            in0=gated[:],
                            in1=bc[:],
                            op=ALU.mult,
                        )

        # ---- phases 3+4: proj+LN1, then transposes/ff1/ff2 interleaved so
        # the LN1 tail hides under ff1(half0) and w2 is SBUF-resident ----
        x1_all = act.tile([128, NQT, E], F32, name="x1_all")
        x1T8 = act.tile([128, NEC, N], F8, name="x1T8")
        gT = act.tile([128, NCT, 512], BF16 if not ff2_fp8 else F8, name="gT")
        with (
            tc.tile_pool(name="xr", bufs=8) as xr_pool,
            tc.tile_pool(name="bcmid", bufs=1) as bcm_pool,
            tc.tile_pool(name="ln_tmp", bufs=2) as ln_pool,
            tc.tile_pool(name="w2res", bufs=1) as w2_pool,
            tc.tile_pool(name="ln2_tmp", bufs=2) as ln2_pool,
            tc.tile_pool(name="out", bufs=2) as out_pool,
        ):
            # residual rows must not queue behind the 4.5MB w2 transfer:
            # issue all of them first (transfers land during attention)
            xrs = []
            for qt in range(NQT):
                xr = xr_pool.tile([128, E], F32, tag="xr", name=f"xr{qt}")
                nc.sync.dma_start(out=xr[:], in_=x_d[qt * 128 : (qt + 1) * 128, :])
                xrs.append(xr)
            w2_sb = w2_pool.tile([128, NCT, E], w2_dt, name="w2_sb")
            nc.sync.dma_start(out=w2_sb[:], in_=w2_d[:])
            bcm = bce = None
            if not (identity_ln and zero_bias):
                bcm = bcm_pool.tile([128, 3, E], F32, tag="bcm", name="bcm")
                for i, d in enumerate((bproj_d, ln1g_d, ln1b_d)):
                    _bcast_dma(nc, bcm[:, i, :], d[0:1, :])
                bce = bcm_pool.tile([128, 3, E], F32, tag="bce", name="bce")
                for i, d in enumerate((bff2_d, ln2g_d, ln2b_d)):
                    _bcast_dma(nc, bce[:, i, :], d[0:1, :])

            with tc.tile_pool(name="y1_ps", bufs=2, space="PSUM") as y1_ps:
                for qt in range(NQT):
                    yp = y1_ps.tile([128, E], F32, tag="yp", name="yp")
                    for hp in range(H // 2):
                        for o, w in ESPL:
                            nc.tensor.matmul(
                                yp[:, o : o + w],
                                og8[:, 2 * hp : 2 * hp + 2, qt * 128 : (qt + 1) * 128],
                                wp_sb[:, hp, :, o : o + w],
                                start=(hp == 0),
                                stop=(hp == H // 2 - 1),
                                perf_mode=DR,
                            )
                    xr = xrs[qt]
                    if not zero_bias:
                        nc.vector.tensor_tensor(out=xr[:], in0=xr[:], in1=bcm[:, 0, :], op=ALU.add)
                    t1 = ln_pool.tile([128, E], F32, tag="lnt1", name="lnt1")
                    scr = ln_pool.tile([128, 8], F32, tag="lnscr", name="lnscr")
                    # guard against accumulate-into-garbage accum_out semantics
                    nc.vector.memset(scr[:, 0:2], 0.0)
                    nc.vector.scalar_tensor_tensor(
                        out=t1[:], in0=yp[:], scalar=PROJ_DESCALE, in1=xr[:],
                        op0=ALU.mult, op1=ALU.add, accum_out=scr[:, 0:1],
                    )
                    _ln_post(nc, ln_pool, t1[:], scr, x1_all[:, qt, :],
                             bcm[:, 1, :] if bcm is not None else None,
                             bcm[:, 2, :] if bcm is not None else None,
                             eps_t, identity_ln, on_scalar=True)

            def emit_tp(tp_ps, qts, scalar_ecs):
                for qt in qts:
                    for ec in range(NEC):
                        pt1 = tp_ps.tile([128, 128], F32, tag="pt1", name="pt1")
                        nc.tensor.transpose(pt1[:], x1_all[:, qt, ec * 128 : (ec + 1) * 128], ident[:])
                        dst = x1T8[:, ec, qt * 128 : (qt + 1) * 128]
                        if ec in scalar_ecs:
                            # Copy lives in every act table: no table-load cost
                            nc.scalar.activation(out=dst, in_=pt1[:], func=AF.Copy, scale=SX)
                        else:
                            nc.vector.tensor_scalar(
                                out=dst, in0=pt1[:], scalar1=SX, scalar2=None, op0=ALU.mult,
                            )

            def emit_ff1(h1_ps, half):
                for g in range(NCT // 2):
                    hp_ = h1_ps.tile([128, 2, 512], F32, tag="h1", name="h1")
                    for j in range(2):
                        ct = 2 * g + j
                        for p in range(NPR):
                            nc.tensor.matmul(
                                hp_[:, j, :],
                                w1_sb[:, ct, p, :, :],
                                x1T8[:, 2 * p : 2 * p + 2, half * 512 : (half + 1) * 512],
                                start=(p == 0),
                                stop=(p == NPR - 1),
                                perf_mode=DR,
                            )
                    if zero_bias:
                        nc.scalar.activation(
                            out=gT[:, 2 * g : 2 * g + 2, :], in_=hp_[:],
                            func=AF.Gelu, scale=FF1_DESCALE,
                        )
                    else:
                        for j in range(2):
                            ct = 2 * g + j
                            nc.scalar.activation(
                                out=gT[:, ct, :], in_=hp_[:, j, :],
                                func=AF.Gelu,
                                bias=bff1_t[:, ct : ct + 1], scale=FF1_DESCALE,
                            )

            def emit_ff2_pass(y2_ps, half, iqs, ln_on_scalar):
                y2p = {iq: y2_ps.tile([128, E], F32, tag=f"y2_{iq % 2}", name=f"y2_{half}_{iq}")
                       for iq in iqs}
                for ct in range(NCT):
                    for iq in iqs:
                        for o, w in ESPL:
                            nc.tensor.matmul(
                                y2p[iq][:, o : o + w],
                                gT[:, ct, iq * 128 : (iq + 1) * 128],
                                w2_sb[:, ct, o : o + w],
                                start=(ct == 0),
                                stop=(ct == NCT - 1),
                            )
                scrs = {}
                for iq in iqs:
                    qt = half * 4 + iq
                    x1q = x1_all[:, qt, :]
                    if not zero_bias:
                        nc.vector.tensor_tensor(out=x1q, in0=x1q, in1=bce[:, 0, :], op=ALU.add)
                    scr = ln2_pool.tile([128, 8], F32, tag="ln2scr", name="ln2scr", bufs=4)
                    scrs[iq] = scr
                    nc.vector.memset(scr[:, 0:2], 0.0)
                    nc.vector.scalar_tensor_tensor(
                        out=x1q, in0=y2p[iq][:],
                        scalar=FF2_DESCALE if ff2_fp8 else 1.0, in1=x1q,
                        op0=ALU.mult, op1=ALU.add, accum_out=scr[:, 0:1],
                    )
                for iq in iqs:
                    qt = half * 4 + iq
                    yout = out_pool.tile([128, E], F32, tag="yout", name="yout")
                    _ln_post(nc, ln2_pool, x1_all[:, qt, :], scrs[iq], yout[:],
                             bce[:, 1, :] if bce is not None else None,
                             bce[:, 2, :] if bce is not None else None,
                             eps_t, identity_ln, on_scalar=ln_on_scalar)
                    nc.sync.dma_start(out=y_d[qt * 128 : (qt + 1) * 128, :], in_=yout[:])

            with (
                tc.tile_pool(name="tp1_ps", bufs=2, space="PSUM") as tp1_ps,
                tc.tile_pool(name="h1a_ps", bufs=3, space="PSUM") as h1a_ps,
            ):
                emit_tp(tp1_ps, range(0, 4), scalar_ecs=())
                emit_ff1(h1a_ps, 0)
            with (
                tc.tile_pool(name="y2a_ps", bufs=1, space="PSUM") as y2a_ps,
                tc.tile_pool(name="tp2_ps", bufs=2, space="PSUM") as tp2_ps,
            ):
                emit_tp(tp2_ps, range(4, 8), scalar_ecs=(0, 2, 4))
                # half0's LN2s execute while ff1(half1) hogs scalar with gelus
                emit_ff2_pass(y2a_ps, 0, (0, 1), ln_on_scalar=False)
                emit_ff2_pass(y2a_ps, 0, (2, 3), ln_on_scalar=False)
            with tc.tile_pool(name="h1b_ps", bufs=3, space="PSUM") as h1b_ps:
                emit_ff1(h1b_ps, 1)
            with tc.tile_pool(name="y2b_ps", bufs=1, space="PSUM") as y2b_ps:
                # single-iq passes: each LN2+DMA chain hides under the next
                # pass's matmuls, leaving only one chain after the last one
                emit_ff2_pass(y2b_ps, 1, (0,), ln_on_scalar=True)
                emit_ff2_pass(y2b_ps, 1, (1,), ln_on_scalar=True)
                emit_ff2_pass(y2b_ps, 1, (2,), ln_on_scalar=True)
                emit_ff2_pass(y2b_ps, 1, (3,), ln_on_scalar=True)

    nc.compile()
    return nc


_NC_CACHE = {}
FF2_FP8 = False


def _get_nc(identity_ln=False, zero_bias=False):
    key = (identity_ln, zero_bias, FF2_FP8)
    if key not in _NC_CACHE:
        _NC_CACHE[key] = _build(identity_ln, zero_bias, FF2_FP8)
    return _NC_CACHE[key]


def _q8(a, scale):
    return (np.asarray(a, np.float32) * scale).astype(ml_dtypes.float8_e4m3)


def _prep_weights(w_qkvr, b_qkvr, w_proj, b_proj, ln1_g, ln1_b,
                  w_ff1, b_ff1, w_ff2, b_ff2, ln2_g, ln2_b):
    w4 = np.asarray(w_qkvr, np.float32).reshape(E, H, D, 4)
    b4 = np.asarray(b_qkvr, np.float32).reshape(H, D, 4)
    s = np.float32(1.0 / np.sqrt(E))

    def head_pairs(w):  # [E, (h d)] -> [128, h, pair, 2, d], fp8*SW
        return np.ascontiguousarray(
            _q8(w, SW).reshape(NPR, 2, 128, H, D).transpose(2, 3, 0, 1, 4)
        )

    wq8 = head_pairs(w4[..., 0].reshape(E, E))
    wk8 = head_pairs(w4[..., 1].reshape(E, E))
    wr8 = head_pairs(w4[..., 3].reshape(E, E))
    wv8 = np.ascontiguousarray(
        _q8(w4[..., 2].reshape(E, E) * s, SWV).reshape(NPR, 2, 128, E).transpose(2, 0, 1, 3)
    )
    wp8 = np.ascontiguousarray(
        _q8(w_proj, SW).reshape(H // 2, 2, D, E).transpose(2, 0, 1, 3)
    )
    w18 = np.ascontiguousarray(
        _q8(w_ff1, SW).reshape(NPR, 2, 128, NCT, 128).transpose(2, 3, 0, 1, 4)
    )
    # raw-scale biases: q/k/r carry 2^13, v carries 2^17
    bqkr = np.ascontiguousarray(
        np.stack([b4[..., 0], b4[..., 1], b4[..., 3]], 0).transpose(2, 0, 1)
    ) * np.float32(SX * SW)
    bv = np.ascontiguousarray((b4[..., 2] * s).reshape(1, E)) * np.float32(SX * SWV * V8_DESCALE)
    bff1 = np.ascontiguousarray(np.asarray(b_ff1, np.float32).reshape(NCT, 128).T)
    if FF2_FP8:
        w2 = _q8(w_ff2, SWV)
    else:
        w2 = np.asarray(w_ff2, np.float32).astype(ml_dtypes.bfloat16)
    w2 = np.ascontiguousarray(w2.reshape(NCT, 128, E).transpose(1, 0, 2))
    return {
        "wq8": wq8, "wk8": wk8, "wr8": wr8, "wv8": wv8, "wp8": wp8, "w18": w18,
        "bqkr": bqkr, "bv": bv,
        "bproj": np.asarray(b_proj, np.float32).reshape(1, E).copy(),
        "ln1g": np.asarray(ln1_g, np.float32).reshape(1, E).copy(),
        "ln1b": np.asarray(ln1_b, np.float32).reshape(1, E).copy(),
        "bff1": bff1, "w2": w2,
        "bff2": np.asarray(b_ff2, np.float32).reshape(1, E).copy(),
        "ln2g": np.asarray(ln2_g, np.float32).reshape(1, E).copy(),
        "ln2b": np.asarray(ln2_b, np.float32).reshape(1, E).copy(),
    }


def _in_maps(inputs):
    x = np.asarray(inputs["x"], np.float32)
    shared = _prep_weights(
        inputs["w_qkvr"], inputs["b_qkvr"], inputs["w_proj"], inputs["b_proj"],
        inputs["ln1_g"], inputs["ln1_b"], inputs["w_ff1"], inputs["b_ff1"],
        inputs["w_ff2"], inputs["b_ff2"], inputs["ln2_g"], inputs["ln2_b"],
    )
    maps = []
    for i in range(N_CORES):
        xi = np.ascontiguousarray(x[i])
        xt8 = np.ascontiguousarray(
            _q8(xi.T, SX).reshape(NEC, 128, N).transpose(1, 0, 2)
        )
        maps.append({**shared, "x": xi, "xt8": xt8})
    return maps


def _flags(inputs):
    z = lambda k: not np.any(np.asarray(inputs[k]))
    one = lambda k: bool(np.all(np.asarray(inputs[k]) == 1.0))
    identity_ln = (one("ln1_g") and z("ln1_b") and one("ln2_g") and z("ln2_b"))
    zero_bias = (z("b_qkvr") and z("b_proj") and z("b_ff2") and z("b_ff1"))
    return identity_ln, zero_bias


def kernel(**inputs) -> np.ndarray:
    identity_ln, zero_bias = _flags(inputs)
    nc = _get_nc(identity_ln, zero_bias)
    res = run_bass_kernel_spmd(nc, _in_maps(inputs), core_ids=list(range(N_CORES)))
    return np.stack([res.results[i]["y"] for i in range(N_CORES)], axis=0)


# revision 42
# speedup vs baseline: 1.5968x; 1.5968x over previous
"""Trainium2 Bass kernel for an encoder block (dense transformer).

Problem: x[8,1024,768]; fused qkvr projection (innermost-4 interleave),
softmax(unscaled logits)/sqrt(768), r-gate, proj + residual + postLN,
exact-gelu FFN (768->3072->768) + residual + postLN.

Strategy: data-parallel over batch - one batch element per NeuronCore,
no collectives. The qkvr/v/proj/ff1 matmuls run in fp8(e4m3) DoubleRow
mode (2x PE rate); ff2 stays bf16 (its quantization error lands
directly on the output and would blow the error budget). Energy and
att@v run bf16. All fp8 operands carry power-of-2 scales folded into
downstream ops for free: exp(scale*energy), gelu(scale*h), and
scalar_tensor_tensor descale-on-evacuate. x arrives pre-transposed and
pre-quantized from the host; softmax uses a ones-column appended to v
to get per-query exp-sums from the same matmul, normalization applied
after att@v. Heads are software-pipelined (qkr of head h+1 issued
between exp and att@v of head h) so the tensor queue never stalls on
the scalar-engine exps.
"""

import sys

if "/opt/trn_rl_repo" not in sys.path:
    sys.path.insert(0, "/opt/trn_rl_repo")

from contextlib import ExitStack

import numpy as np
import ml_dtypes

import concourse.bass as bass
import concourse.mybir as mybir
import concourse.tile as tile
from concourse import bacc
from concourse.bass_utils import run_bass_kernel_spmd
from concourse.masks import make_identity

F32 = mybir.dt.float32
F32R = mybir.dt.float32r
BF16 = mybir.dt.bfloat16
F8 = mybir.dt.float8e4
F8E5 = mybir.dt.float8e5
AF = mybir.ActivationFunctionType
ALU = mybir.AluOpType
DR = mybir.MatmulPerfMode.DoubleRow

N_CORES = 8
B, N, E = 8, 1024, 768
H, D = 8, 96          # heads, head dim
C = 4 * E             # ffn hidden 3072
NQT = N // 128        # 8 seq tiles
NEC = E // 128        # 6 embedding chunks
NPR = NEC // 2        # 3 e-chunk pairs (DoubleRow)
NCT = C // 128        # 24 ffn chunks
LN_EPS = 1e-5
ESPL = [(0, 512), (512, 256)]  # bank-aligned 768 split for psum outputs

# power-of-2 fp8 scales; all descales fold into existing ops
SX = 16.0            # x and x1 activations (2^4)
SW = 512.0           # wq/wk/wr/wproj/wff1 (2^9)
SWV = 8192.0         # wv (1/sqrt(E) pre-folded, needs more lift) (2^13)
V8_DESCALE = 1.0 / 2.0**11   # v psum (2^17) -> e4m3 v_aug at v_true*2^6
C1 = 128.0           # ones-column in v_aug (must fit e4m3)
SUMS_SCALE = 16.0    # sums staging lift so og8 = og_true * 2^8
EXP_OFF = -12.0      # global exp offset: softmax-invariant, fits e4m3 window
EXP_SCALE = 1.0 / (SX * SW) ** 2       # 2^-26
PROJ_DESCALE = 1.0 / (2.0**8 * SW)     # 2^-17
FF1_DESCALE = 1.0 / (SX * SW)          # 2^-13
FF2_DESCALE = 1.0 / SWV                # 2^-13 (ff2_fp8 variant only)


def _bcast_dma(nc, out_ap, row_ap):
    """Replicate a DRAM row across partitions (partition-step-0 source)."""
    src = bass.AP(
        tensor=row_ap.tensor,
        offset=row_ap.offset,
        ap=[[0, out_ap.shape[0]], list(row_ap.ap[-1])],
    )
    nc.gpsimd.dma_start(out=out_ap, in_=src)


def _ln_post(nc, pool, t1, scr, dst, g_bc, b_bc, eps_t, identity_ln, on_scalar):
    """LN(t1) -> dst. scr[:,0:1] must already hold sum(t1) (producer's
    accum_out). Square/Identity live in every act table (no table loads);
    stats come from row sums instead of bn_stats to keep vector light.
    on_scalar picks the engine for the two [128,E] passes (square, apply)
    so callers can balance scalar vs vector load per phase."""
    s1, s2, m, msq, v, sq, r, nb = (scr[:, i : i + 1] for i in range(8))
    junk = pool.tile([128, E], F32, tag="lnjunk", name="lnjunk", bufs=1)
    if on_scalar:
        nc.scalar.activation(out=junk[:], in_=t1, func=AF.Square, accum_out=s2)
    else:
        nc.vector.scalar_tensor_tensor(
            out=junk[:], in0=t1, scalar=1.0, in1=t1,
            op0=ALU.mult, op1=ALU.mult, accum_out=s2,
        )
    nc.vector.tensor_scalar(out=m, in0=s1, scalar1=1.0 / E, scalar2=None, op0=ALU.mult)
    nc.vector.tensor_tensor(out=msq, in0=m, in1=m, op=ALU.mult)
    nc.vector.scalar_tensor_tensor(
        out=v, in0=s2, scalar=1.0 / E, in1=msq, op0=ALU.mult, op1=ALU.subtract,
    )
    nc.scalar.activation(out=sq, in_=v, func=AF.Sqrt, bias=eps_t[:], scale=1.0)
    nc.vector.reciprocal(r, sq)
    if on_scalar:
        nc.vector.tensor_scalar(out=nb, in0=m, scalar1=r, scalar2=-1.0, op0=ALU.mult, op1=ALU.mult)
    t2a = dst if identity_ln else pool.tile([128, E], F32, tag="lnt2", name="lnt2")[:]
    if on_scalar:
        nc.scalar.activation(out=t2a, in_=t1, func=AF.Identity, bias=nb, scale=r)
    else:
        nc.vector.tensor_scalar(
            out=t2a, in0=t1, scalar1=m, scalar2=r, op0=ALU.subtract, op1=ALU.mult,
        )
    if not identity_ln:
        nc.vector.tensor_tensor(out=t2a, in0=t2a, in1=g_bc, op=ALU.mult)
        nc.vector.tensor_tensor(out=dst, in0=t2a, in1=b_bc, op=ALU.add)


def _build(identity_ln=False, zero_bias=False, ff2_fp8=False):
    nc = bacc.Bacc(num_devices=N_CORES)

    x_d = nc.declare_dram_parameter("x", [N, E], F32, isOutput=False)
    xt8_d = nc.declare_dram_parameter("xt8", [128, NEC, N], F8, isOutput=False)
    wq8_d = nc.declare_dram_parameter("wq8", [128, H, NPR, 2, D], F8, isOutput=False)
    wk8_d = nc.declare_dram_parameter("wk8", [128, H, NPR, 2, D], F8, isOutput=False)
    wr8_d = nc.declare_dram_parameter("wr8", [128, H, NPR, 2, D], F8, isOutput=False)
    wv8_d = nc.declare_dram_parameter("wv8", [128, NPR, 2, E], F8, isOutput=False)
    wp8_d = nc.declare_dram_parameter("wp8", [D, H // 2, 2, E], F8, isOutput=False)
    w18_d = nc.declare_dram_parameter("w18", [128, NCT, NPR, 2, 128], F8, isOutput=False)
    bqkr_d = nc.declare_dram_parameter("bqkr", [D, 3, H], F32, isOutput=False)
    bv_d = nc.declare_dram_parameter("bv", [1, E], F32, isOutput=False)
    bproj_d = nc.declare_dram_parameter("bproj", [1, E], F32, isOutput=False)
    ln1g_d = nc.declare_dram_parameter("ln1g", [1, E], F32, isOutput=False)
    ln1b_d = nc.declare_dram_parameter("ln1b", [1, E], F32, isOutput=False)
    bff1_d = nc.declare_dram_parameter("bff1", [128, NCT], F32, isOutput=False)
    w2_dt = F8 if ff2_fp8 else BF16
    w2_d = nc.declare_dram_parameter("w2", [128, NCT, E], w2_dt, isOutput=False)
    bff2_d = nc.declare_dram_parameter("bff2", [1, E], F32, isOutput=False)
    ln2g_d = nc.declare_dram_parameter("ln2g", [1, E], F32, isOutput=False)
    ln2b_d = nc.declare_dram_parameter("ln2b", [1, E], F32, isOutput=False)
    y_d = nc.declare_dram_parameter("y", [N, E], F32, isOutput=True)

    with tile.TileContext(nc) as tc, ExitStack() as ctx:
        # ---- whole-kernel pools ----
        persist = ctx.enter_context(tc.tile_pool(name="persist", bufs=1))
        wts = ctx.enter_context(tc.tile_pool(name="wts", bufs=1))
        act = ctx.enter_context(tc.tile_pool(name="act", bufs=1))

        ident = persist.tile([128, 128], F32)
        make_identity(nc, ident[:])
        eps_t = persist.tile([128, 1], F32)
        nc.vector.memset(eps_t[:], LN_EPS)
        expoff_t = persist.tile([128, 1], F32)
        nc.vector.memset(expoff_t[:], EXP_OFF)

        # ---- weight/x DMAs, transfer order = issue order: qkr(h0) operands
        # first so the tensor engine starts ~3us in ----
        xT8 = act.tile([128, NEC, N], F8, name="xT8")
        nc.sync.dma_start(out=xT8[:, :, 0:512], in_=xt8_d[:, :, 0:512])
        wq_sb = wts.tile([128, H, NPR, 2, D], F8, name="wq_sb")
        wk_sb = wts.tile([128, H, NPR, 2, D], F8, name="wk_sb")
        wr_sb = wts.tile([128, H, NPR, 2, D], F8, name="wr_sb")
        wv_sb = wts.tile([128, NPR, 2, E], F8, name="wv_sb")
        nc.sync.dma_start(out=wq_sb[:], in_=wq8_d[:])
        nc.sync.dma_start(out=wv_sb[:], in_=wv8_d[:])
        nc.sync.dma_start(out=wk_sb[:], in_=wk8_d[:])
        nc.sync.dma_start(out=wr_sb[:], in_=wr8_d[:])
        nc.sync.dma_start(out=xT8[:, :, 512:1024], in_=xt8_d[:, :, 512:1024])
        wp_sb = wts.tile([D, H // 2, 2, E], F8, name="wp_sb")
        nc.sync.dma_start(out=wp_sb[:], in_=wp8_d[:])
        # bulk weights last on the same queue: they transfer only after the
        # attention-critical items above, but well before ff1 needs them
        w1_sb = wts.tile([128, NCT, NPR, 2, 128], F8, name="w1_sb")
        nc.sync.dma_start(out=w1_sb[:], in_=w18_d[:])
        bff1_t = wts.tile([128, NCT], F32, name="bff1_t")
        nc.sync.dma_start(out=bff1_t[:], in_=bff1_d[:])
        bqkr_t = persist.tile([D, 3, H], F32)
        nc.sync.dma_start(out=bqkr_t[:], in_=bqkr_d[:])

        # warm-up matmuls (PE clock ramp)
        warm_t = persist.tile([128, 128], BF16)
        nc.vector.memset(warm_t[:], 0.0)
        with tc.tile_pool(name="warm_ps", bufs=2, space="PSUM") as warm_ps:
            for _ in range(12):
                wp_ = warm_ps.tile([128, 128], F32, tag="wp_", name="wp_")
                nc.tensor.matmul(wp_[:], warm_t[:], warm_t[:], start=True, stop=True)
                nc.tensor.matmul(wp_[:], warm_t[:], warm_t[:], start=True, stop=True)

        # ---- phases 1+2: v projection and attention share the qkr pools so
        # qkr(head 0) can be emitted before the v loop (fills the DMA wait) ----
        # last dim padded 97->128: dual-fp8 LDWEIGHTS requires full 32-aligned
        # chunk geometry; pad columns stay zero and emit into unused psum rows
        DP = 128
        v_aug = act.tile([128, H, NQT, DP], F8, name="v_aug")
        nc.vector.memset(v_aug[:, :, :, D : D + 1], C1)
        nc.vector.memset(v_aug[:, :, :, D + 1 : DP], 0.0)
        og8 = act.tile([D, H, N], F8, name="og8")  # gated out, scale 2^8
        with (
            tc.tile_pool(name="qkr", bufs=2) as qkr_pool,
            tc.tile_pool(name="expET", bufs=2) as exp_pool,
            tc.tile_pool(name="att_tmp", bufs=2) as tmp_pool,
            tc.tile_pool(name="qkr_ps", bufs=2, space="PSUM") as qkr_ps,
        ):
            def emit_qkr(h):
                dst = qkr_pool.tile([D, 3, N], BF16, tag="qkrT", name=f"qkrT{h}")
                for qh in range(2):
                    for si, w_sb in enumerate((wq_sb, wk_sb, wr_sb)):
                        qp = qkr_ps.tile([D, 512], F32, tag="qp", name="qp")
                        for p in range(NPR):
                            nc.tensor.matmul(
                                qp[:],
                                w_sb[:, h, p, :, :],
                                xT8[:, 2 * p : 2 * p + 2, qh * 512 : (qh + 1) * 512],
                                start=(p == 0),
                                stop=(p == NPR - 1),
                                perf_mode=DR,
                            )
                        dslc = dst[:, si, qh * 512 : (qh + 1) * 512]
                        if zero_bias:
                            nc.vector.tensor_copy(dslc, qp[:])
                        else:
                            nc.vector.tensor_scalar(
                                out=dslc, in0=qp[:],
                                scalar1=bqkr_t[:, si, h : h + 1], scalar2=None,
                                op0=ALU.add,
                            )
                return dst

            with tc.tile_pool(name="v_ps", bufs=2, space="PSUM") as v_ps:
                qkrT = emit_qkr(0)
                bv_bc = None
                if not zero_bias:
                    bv_bc = tmp_pool.tile([128, E], F32, tag="bv", name="bv_bc", bufs=1)
                    _bcast_dma(nc, bv_bc[:], bv_d[0:1, :])
                for qt in range(NQT):
                    vp = v_ps.tile([128, E], F32, tag="vp", name="vp")
                    for p in range(NPR):
                        for o, w in ESPL:
                            nc.tensor.matmul(
                                vp[:, o : o + w],
                                xT8[:, 2 * p : 2 * p + 2, qt * 128 : (qt + 1) * 128],
                                wv_sb[:, p, :, o : o + w],
                                start=(p == 0),
                                stop=(p == NPR - 1),
                                perf_mode=DR,
                            )
                    vph = vp.rearrange("k (h d) -> k h d", h=H)
                    if zero_bias:
                        nc.vector.tensor_scalar(
                            out=v_aug[:, :, qt, 0:D], in0=vph,
                            scalar1=V8_DESCALE, scalar2=None, op0=ALU.mult,
                        )
                    else:
                        nc.vector.scalar_tensor_tensor(
                            out=v_aug[:, :, qt, 0:D],
                            in0=vph, scalar=V8_DESCALE,
                            in1=bv_bc.rearrange("k (h d) -> k h d", h=H),
                            op0=ALU.mult, op1=ALU.add,
                        )
            with (
                tc.tile_pool(name="eng_ps", bufs=2, space="PSUM") as eng_ps,
                tc.tile_pool(name="att_ps", bufs=2, space="PSUM") as att_ps,
            ):
                for h in range(H):
                    cur = qkrT
                    expET = exp_pool.tile([128, NQT, N], F8, tag="expET", name=f"expET{h}")
                    for qh in range(2):
                        for g in range(4):
                            ep = eng_ps.tile([128, 2, 512], F32, tag="ep", name="ep")
                            for j in range(2):
                                kt = 2 * g + j
                                nc.tensor.matmul(
                                    ep[:, j, :],
                                    cur[:, 1, kt * 128 : (kt + 1) * 128],
                                    cur[:, 0, qh * 512 : (qh + 1) * 512],
                                    start=True,
                                    stop=True,
                                )
                            nc.scalar.activation(
                                out=expET[:, 2 * g : 2 * g + 2, qh * 512 : (qh + 1) * 512],
                                in_=ep[:],
                                func=AF.Exp,
                                scale=EXP_SCALE,
                                bias=expoff_t[:],
                            )
                    if h + 1 < H:
                        qkrT = emit_qkr(h + 1)
                    for qh in range(2):
                        op_ = att_ps.tile([128, 512], F32, tag="op", name="op")
                        for t2 in range(NQT // 2):
                            nc.tensor.matmul(
                                op_[:],
                                v_aug[:, h, 2 * t2 : 2 * t2 + 2, :],
                                expET[:, 2 * t2 : 2 * t2 + 2, qh * 512 : (qh + 1) * 512],
                                start=(t2 == 0),
                                stop=(t2 == NQT // 2 - 1),
                                perf_mode=DR,
                            )
                        sums = tmp_pool.tile([1, 512], F32, tag="sums", name="sums")
                        nc.vector.tensor_scalar(
                            out=sums[:], in0=op_[D : D + 1, :],
                            scalar1=SUMS_SCALE, scalar2=None, op0=ALU.mult,
                        )
                        recip = tmp_pool.tile([1, 512], F32, tag="recip", name="recip")
                        nc.vector.reciprocal_approx_fast(recip[:], sums[:])
                        bc = tmp_pool.tile([D, 512], F32, tag="bc", name="bc")
                        nc.gpsimd.partition_broadcast(bc[:], recip[:])
                        gated = tmp_pool.tile([D, 512], F32, tag="gated", name="gated")
                        nc.vector.tensor_tensor(
                            out=gated[:],
                            in0=op_[0:D, :],
                            in1=cur[:, 2, qh * 512 : (qh + 1) * 512],
                            op=ALU.mult,
                        )
                        nc.vector.tensor_tensor(
                            out=og8[:, h, qh * 512 : (qh + 1) * 512],
                            in0=gated[:],
                            in1=bc[:],
                            op=ALU.mult,
                        )

        # ---- phases 3+4: proj+LN1, then transposes/ff1/ff2 interleaved so
        # the LN1 tail hides under ff1(half0) and w2 is SBUF-resident ----
        x1_all = act.tile([128, NQT, E], F32, name="x1_all")
        x1T8 = act.tile([128, NEC, N], F8, name="x1T8")
        gT = act.tile([128, NCT, 512], BF16 if not ff2_fp8 else F8, name="gT")
        with (
            tc.tile_pool(name="xr", bufs=8) as xr_pool,
            tc.tile_pool(name="bcmid", bufs=1) as bcm_pool,
            tc.tile_pool(name="ln_tmp", bufs=2) as ln_pool,
            tc.tile_pool(name="w2res", bufs=1) as w2_pool,
            tc.tile_pool(name="ln2_tmp", bufs=2) as ln2_pool,
            tc.tile_pool(name="out", bufs=2) as out_pool,
        ):
            # residual rows must not queue behind the 4.5MB w2 transfer:
            # issue all of them first (transfers land during attention)
            xrs = []
            for qt in range(NQT):
                xr = xr_pool.tile([128, E], F32, tag="xr", name=f"xr{qt}")
                nc.sync.dma_start(out=xr[:], in_=x_d[qt * 128 : (qt + 1) * 128, :])
                xrs.append(xr)
            w2_sb = w2_pool.tile([128, NCT, E], w2_dt, name="w2_sb")
            nc.sync.dma_start(out=w2_sb[:], in_=w2_d[:])
            bcm = bce = None
            if not (identity_ln and zero_bias):
                bcm = bcm_pool.tile([128, 3, E], F32, tag="bcm", name="bcm")
                for i, d in enumerate((bproj_d, ln1g_d, ln1b_d)):
                    _bcast_dma(nc, bcm[:, i, :], d[0:1, :])
                bce = bcm_pool.tile([128, 3, E], F32, tag="bce", name="bce")
                for i, d in enumerate((bff2_d, ln2g_d, ln2b_d)):
                    _bcast_dma(nc, bce[:, i, :], d[0:1, :])

            with tc.tile_pool(name="y1_ps", bufs=2, space="PSUM") as y1_ps:
                for qt in range(NQT):
                    yp = y1_ps.tile([128, E], F32, tag="yp", name="yp")
                    for hp in range(H // 2):
                        for o, w in ESPL:
                            nc.tensor.matmul(
                                yp[:, o : o + w],
                                og8[:, 2 * hp : 2 * hp + 2, qt * 128 : (qt + 1) * 128],
                                wp_sb[:, hp, :, o : o + w],
                                start=(hp == 0),
                                stop=(hp == H // 2 - 1),
                                perf_mode=DR,
                            )
                    xr = xrs[qt]
                    if not zero_bias:
                        nc.vector.tensor_tensor(out=xr[:], in0=xr[:], in1=bcm[:, 0, :], op=ALU.add)
                    t1 = ln_pool.tile([128, E], F32, tag="lnt1", name="lnt1")
                    scr = ln_pool.tile([128, 8], F32, tag="lnscr", name="lnscr")
                    # guard against accumulate-into-garbage accum_out semantics
                    nc.vector.memset(scr[:, 0:2], 0.0)
                    nc.vector.scalar_tensor_tensor(
                        out=t1[:], in0=yp[:], scalar=PROJ_DESCALE, in1=xr[:],
                        op0=ALU.mult, op1=ALU.add, accum_out=scr[:, 0:1],
                    )
                    _ln_post(nc, ln_pool, t1[:], scr, x1_all[:, qt, :],
                             bcm[:, 1, :] if bcm is not None else None,
                             bcm[:, 2, :] if bcm is not None else None,
                             eps_t, identity_ln, on_scalar=True)

            def emit_tp(tp_ps, qts, scalar_ecs):
                for qt in qts:
                    for ec in range(NEC):
                        pt1 = tp_ps.tile([128, 128], F32, tag="pt1", name="pt1")
                        nc.tensor.transpose(pt1[:], x1_all[:, qt, ec * 128 : (ec + 1) * 128], ident[:])
                        dst = x1T8[:, ec, qt * 128 : (qt + 1) * 128]
                        if ec in scalar_ecs:
                            # Copy lives in every act table: no table-load cost
                            nc.scalar.activation(out=dst, in_=pt1[:], func=AF.Copy, scale=SX)
                        else:
                            nc.vector.tensor_scalar(
                                out=dst, in0=pt1[:], scalar1=SX, scalar2=None, op0=ALU.mult,
                            )

            def emit_ff1(h1_ps, half):
                for g in range(NCT // 2):
                    hp_ = h1_ps.tile([128, 2, 512], F32, tag="h1", name="h1")
                    for j in range(2):
                        ct = 2 * g + j
                        for p in range(NPR):
                            nc.tensor.matmul(
                                hp_[:, j, :],
                                w1_sb[:, ct, p, :, :],
                                x1T8[:, 2 * p : 2 * p + 2, half * 512 : (half + 1) * 512],
                                start=(p == 0),
                                stop=(p == NPR - 1),
                                perf_mode=DR,
                            )
                    if zero_bias:
                        nc.scalar.activation(
                            out=gT[:, 2 * g : 2 * g + 2, :], in_=hp_[:],
                            func=AF.Gelu, scale=FF1_DESCALE,
                        )
                    else:
                        for j in range(2):
                            ct = 2 * g + j
                            nc.scalar.activation(
                                out=gT[:, ct, :], in_=hp_[:, j, :],
                                func=AF.Gelu,
                                bias=bff1_t[:, ct : ct + 1], scale=FF1_DESCALE,
                            )

            def emit_ff2_pass(y2_ps, half, iqs, ln_on_scalar):
                y2p = {iq: y2_ps.tile([128, E], F32, tag=f"y2_{iq % 2}", name=f"y2_{half}_{iq}")
                       for iq in iqs}
                for ct in range(NCT):
                    for iq in iqs:
                        for o, w in ESPL:
                            nc.tensor.matmul(
                                y2p[iq][:, o : o + w],
                                gT[:, ct, iq * 128 : (iq + 1) * 128],
                                w2_sb[:, ct, o : o + w],
                                start=(ct == 0),
                                stop=(ct == NCT - 1),
                            )
                scrs = {}
                for iq in iqs:
                    qt = half * 4 + iq
                    x1q = x1_all[:, qt, :]
                    if not zero_bias:
                        nc.vector.tensor_tensor(out=x1q, in0=x1q, in1=bce[:, 0, :], op=ALU.add)
                    scr = ln2_pool.tile([128, 8], F32, tag="ln2scr", name="ln2scr", bufs=4)
                    scrs[iq] = scr
                    nc.vector.memset(scr[:, 0:2], 0.0)
                    nc.vector.scalar_tensor_tensor(
                        out=x1q, in0=y2p[iq][:],
                        scalar=FF2_DESCALE if ff2_fp8 else 1.0, in1=x1q,
                        op0=ALU.mult, op1=ALU.add, accum_out=scr[:, 0:1],
                    )
                for iq in iqs:
                    qt = half * 4 + iq
                    yout = out_pool.tile([128, E], F32, tag="yout", name="yout")
                    _ln_post(nc, ln2_pool, x1_all[:, qt, :], scrs[iq], yout[:],
                             bce[:, 1, :] if bce is not None else None,
                             bce[:, 2, :] if bce is not None else None,
                             eps_t, identity_ln, on_scalar=ln_on_scalar)
                    nc.sync.dma_start(out=y_d[qt * 128 : (qt + 1) * 128, :], in_=yout[:])

            with (
                tc.tile_pool(name="tp1_ps", bufs=2, space="PSUM") as tp1_ps,
                tc.tile_pool(name="h1a_ps", bufs=3, space="PSUM") as h1a_ps,
            ):
                emit_tp(tp1_ps, range(0, 4), scalar_ecs=())
                emit_ff1(h1a_ps, 0)
            with (
                tc.tile_pool(name="y2a_ps", bufs=1, space="PSUM") as y2a_ps,
                tc.tile_pool(name="tp2_ps", bufs=2, space="PSUM") as tp2_ps,
            ):
                emit_tp(tp2_ps, range(4, 8), scalar_ecs=(0, 2, 4))
                # half0's LN2s execute while ff1(half1) hogs scalar with gelus
                emit_ff2_pass(y2a_ps, 0, (0, 1), ln_on_scalar=False)
                emit_ff2_pass(y2a_ps, 0, (2, 3), ln_on_scalar=False)
            with tc.tile_pool(name="h1b_ps", bufs=3, space="PSUM") as h1b_ps:
                emit_ff1(h1b_ps, 1)
            with tc.tile_pool(name="y2b_ps", bufs=1, space="PSUM") as y2b_ps:
                # single-iq passes: each LN2+DMA chain hides under the next
                # pass's matmuls, leaving only one chain after the last one
                emit_ff2_pass(y2b_ps, 1, (0,), ln_on_scalar=True)
                emit_ff2_pass(y2b_ps, 1, (1,), ln_on_scalar=True)
                emit_ff2_pass(y2b_ps, 1, (2,), ln_on_scalar=True)
                emit_ff2_pass(y2b_ps, 1, (3,), ln_on_scalar=True)

    nc.compile()
    return nc


_NC_CACHE = {}
FF2_FP8 = False


def _get_nc(identity_ln=False, zero_bias=False):
    key = (identity_ln, zero_bias, FF2_FP8)
    if key not in _NC_CACHE:
        _NC_CACHE[key] = _build(identity_ln, zero_bias, FF2_FP8)
    return _NC_CACHE[key]


def _q8(a, scale):
    return (np.asarray(a, np.float32) * scale).astype(ml_dtypes.float8_e4m3)


def _prep_weights(w_qkvr, b_qkvr, w_proj, b_proj, ln1_g, ln1_b,
                  w_ff1, b_ff1, w_ff2, b_ff2, ln2_g, ln2_b):
    w4 = np.asarray(w_qkvr, np.float32).reshape(E, H, D, 4)
    b4 = np.asarray(b_qkvr, np.float32).reshape(H, D, 4)
    s = np.float32(1.0 / np.sqrt(E))

    def head_pairs(w):  # [E, (h d)] -> [128, h, pair, 2, d], fp8*SW
        return np.ascontiguousarray(
            _q8(w, SW).reshape(NPR, 2, 128, H, D).transpose(2, 3, 0, 1, 4)
        )

    wq8 = head_pairs(w4[..., 0].reshape(E, E))
    wk8 = head_pairs(w4[..., 1].reshape(E, E))
    wr8 = head_pairs(w4[..., 3].reshape(E, E))
    wv8 = np.ascontiguousarray(
        _q8(w4[..., 2].reshape(E, E) * s, SWV).reshape(NPR, 2, 128, E).transpose(2, 0, 1, 3)
    )
    wp8 = np.ascontiguousarray(
        _q8(w_proj, SW).reshape(H // 2, 2, D, E).transpose(2, 0, 1, 3)
    )
    w18 = np.ascontiguousarray(
        _q8(w_ff1, SW).reshape(NPR, 2, 128, NCT, 128).transpose(2, 3, 0, 1, 4)
    )
    # raw-scale biases: q/k/r carry 2^13, v carries 2^17
    bqkr = np.ascontiguousarray(
        np.stack([b4[..., 0], b4[..., 1], b4[..., 3]], 0).transpose(2, 0, 1)
    ) * np.float32(SX * SW)
    bv = np.ascontiguousarray((b4[..., 2] * s).reshape(1, E)) * np.float32(SX * SWV * V8_DESCALE)
    bff1 = np.ascontiguousarray(np.asarray(b_ff1, np.float32).reshape(NCT, 128).T)
    if FF2_FP8:
        w2 = _q8(w_ff2, SWV)
    else:
        w2 = np.asarray(w_ff2, np.float32).astype(ml_dtypes.bfloat16)
    w2 = np.ascontiguousarray(w2.reshape(NCT, 128, E).transpose(1, 0, 2))
    return {
        "wq8": wq8, "wk8": wk8, "wr8": wr8, "wv8": wv8, "wp8": wp8, "w18": w18,
        "bqkr": bqkr, "bv": bv,
        "bproj": np.asarray(b_proj, np.float32).reshape(1, E).copy(),
        "ln1g": np.asarray(ln1_g, np.float32).reshape(1, E).copy(),
        "ln1b": np.asarray(ln1_b, np.float32).reshape(1, E).copy(),
        "bff1": bff1, "w2": w2,
        "bff2": np.asarray(b_ff2, np.float32).reshape(1, E).copy(),
        "ln2g": np.asarray(ln2_g, np.float32).reshape(1, E).copy(),
        "ln2b": np.asarray(ln2_b, np.float32).reshape(1, E).copy(),
    }


def _in_maps(inputs):
    x = np.asarray(inputs["x"], np.float32)
    shared = _prep_weights(
        inputs["w_qkvr"], inputs["b_qkvr"], inputs["w_proj"], inputs["b_proj"],
        inputs["ln1_g"], inputs["ln1_b"], inputs["w_ff1"], inputs["b_ff1"],
        inputs["w_ff2"], inputs["b_ff2"], inputs["ln2_g"], inputs["ln2_b"],
    )
    maps = []
    for i in range(N_CORES):
        xi = np.ascontiguousarray(x[i])
        xt8 = np.ascontiguousarray(
            _q8(xi.T, SX).reshape(NEC, 128, N).transpose(1, 0, 2)
        )
        maps.append({**shared, "x": xi, "xt8": xt8})
    return maps


def _flags(inputs):
    z = lambda k: not np.any(np.asarray(inputs[k]))
    one = lambda k: bool(np.all(np.asarray(inputs[k]) == 1.0))
    identity_ln = (one("ln1_g") and z("ln1_b") and one("ln2_g") and z("ln2_b"))
    zero_bias = (z("b_qkvr") and z("b_proj") and z("b_ff2") and z("b_ff1"))
    return identity_ln, zero_bias


def kernel(**inputs) -> np.ndarray:
    identity_ln, zero_bias = _flags(inputs)
    nc = _get_nc(identity_ln, zero_bias)
    res = run_bass_kernel_spmd(nc, _in_maps(inputs), core_ids=list(range(N_CORES)))
    return np.stack([res.results[i]["y"] for i in range(N_CORES)], axis=0)


# revision 44
# speedup vs baseline: 1.5987x; 1.0012x over previous
"""Trainium2 Bass kernel for an encoder block (dense transformer).

Problem: x[8,1024,768]; fused qkvr projection (innermost-4 interleave),
softmax(unscaled logits)/sqrt(768), r-gate, proj + residual + postLN,
exact-gelu FFN (768->3072->768) + residual + postLN.

Strategy: data-parallel over batch - one batch element per NeuronCore,
no collectives. The qkvr/v/proj/ff1 matmuls run in fp8(e4m3) DoubleRow
mode (2x PE rate); ff2 stays bf16 (its quantization error lands
directly on the output and would blow the error budget). Energy and
att@v run bf16. All fp8 operands carry power-of-2 scales folded into
downstream ops for free: exp(scale*energy), gelu(scale*h), and
scalar_tensor_tensor descale-on-evacuate. x arrives pre-transposed and
pre-quantized from the host; softmax uses a ones-column appended to v
to get per-query exp-sums from the same matmul, normalization applied
after att@v. Heads are software-pipelined (qkr of head h+1 issued
between exp and att@v of head h) so the tensor queue never stalls on
the scalar-engine exps.
"""

import sys

if "/opt/trn_rl_repo" not in sys.path:
    sys.path.insert(0, "/opt/trn_rl_repo")

from contextlib import ExitStack

import numpy as np
import ml_dtypes

import concourse.bass as bass
import concourse.mybir as mybir
import concourse.tile as tile
from concourse import bacc
from concourse.bass_utils import run_bass_kernel_spmd
from concourse.masks import make_identity

F32 = mybir.dt.float32
F32R = mybir.dt.float32r
BF16 = mybir.dt.bfloat16
F8 = mybir.dt.float8e4
F8E5 = mybir.dt.float8e5
AF = mybir.ActivationFunctionType
ALU = mybir.AluOpType
DR = mybir.MatmulPerfMode.DoubleRow

N_CORES = 8
B, N, E = 8, 1024, 768
H, D = 8, 96          # heads, head dim
C = 4 * E             # ffn hidden 3072
NQT = N // 128        # 8 seq tiles
NEC = E // 128        # 6 embedding chunks
NPR = NEC // 2        # 3 e-chunk pairs (DoubleRow)
NCT = C // 128        # 24 ffn chunks
LN_EPS = 1e-5
ESPL = [(0, 512), (512, 256)]  # bank-aligned 768 split for psum outputs

# power-of-2 fp8 scales; all descales fold into existing ops
SX = 16.0            # x and x1 activations (2^4)
SW = 512.0           # wq/wk/wr/wproj/wff1 (2^9)
SWV = 8192.0         # wv (1/sqrt(E) pre-folded, needs more lift) (2^13)
V8_DESCALE = 1.0 / 2.0**11   # v psum (2^17) -> e4m3 v_aug at v_true*2^6
C1 = 128.0           # ones-column in v_aug (must fit e4m3)
SUMS_SCALE = 16.0    # sums staging lift so og8 = og_true * 2^8
EXP_OFF = -12.0      # global exp offset: softmax-invariant, fits e4m3 window
EXP_SCALE = 1.0 / (SX * SW) ** 2       # 2^-26
PROJ_DESCALE = 1.0 / (2.0**8 * SW)     # 2^-17
FF1_DESCALE = 1.0 / (SX * SW)          # 2^-13
FF2_DESCALE = 1.0 / SWV                # 2^-13 (ff2_fp8 variant only)


def _bcast_dma(nc, out_ap, row_ap):
    """Replicate a DRAM row across partitions (partition-step-0 source)."""
    src = bass.AP(
        tensor=row_ap.tensor,
        offset=row_ap.offset,
        ap=[[0, out_ap.shape[0]], list(row_ap.ap[-1])],
    )
    nc.gpsimd.dma_start(out=out_ap, in_=src)


def _ln_post(nc, pool, t1, scr, dst, g_bc, b_bc, eps_t, identity_ln, on_scalar):
    """LN(t1) -> dst. scr[:,0:1] must already hold sum(t1) (producer's
    accum_out). Square/Identity live in every act table (no table loads);
    stats come from row sums instead of bn_stats to keep vector light.
    on_scalar picks the engine for the two [128,E] passes (square, apply)
    so callers can balance scalar vs vector load per phase."""
    s1, s2, m, msq, v, sq, r, nb = (scr[:, i : i + 1] for i in range(8))
    junk = pool.tile([128, E], F32, tag="lnjunk", name="lnjunk", bufs=1)
    if on_scalar:
        nc.scalar.activation(out=junk[:], in_=t1, func=AF.Square, accum_out=s2)
    else:
        nc.vector.scalar_tensor_tensor(
            out=junk[:], in0=t1, scalar=1.0, in1=t1,
            op0=ALU.mult, op1=ALU.mult, accum_out=s2,
        )
    nc.vector.tensor_scalar(out=m, in0=s1, scalar1=1.0 / E, scalar2=None, op0=ALU.mult)
    nc.vector.tensor_tensor(out=msq, in0=m, in1=m, op=ALU.mult)
    nc.vector.scalar_tensor_tensor(
        out=v, in0=s2, scalar=1.0 / E, in1=msq, op0=ALU.mult, op1=ALU.subtract,
    )
    nc.scalar.activation(out=sq, in_=v, func=AF.Sqrt, bias=eps_t[:], scale=1.0)
    nc.vector.reciprocal(r, sq)
    if on_scalar:
        nc.vector.tensor_scalar(out=nb, in0=m, scalar1=r, scalar2=-1.0, op0=ALU.mult, op1=ALU.mult)
    t2a = dst if identity_ln else pool.tile([128, E], F32, tag="lnt2", name="lnt2")[:]
    if on_scalar:
        nc.scalar.activation(out=t2a, in_=t1, func=AF.Identity, bias=nb, scale=r)
    else:
        nc.vector.tensor_scalar(
            out=t2a, in0=t1, scalar1=m, scalar2=r, op0=ALU.subtract, op1=ALU.mult,
        )
    if not identity_ln:
        nc.vector.tensor_tensor(out=t2a, in0=t2a, in1=g_bc, op=ALU.mult)
        nc.vector.tensor_tensor(out=dst, in0=t2a, in1=b_bc, op=ALU.add)


def _build(identity_ln=False, zero_bias=False, ff2_fp8=False):
    nc = bacc.Bacc(num_devices=N_CORES)

    x_d = nc.declare_dram_parameter("x", [N, E], F32, isOutput=False)
    xt8_d = nc.declare_dram_parameter("xt8", [128, NEC, N], F8, isOutput=False)
    wq8_d = nc.declare_dram_parameter("wq8", [128, H, NPR, 2, D], F8, isOutput=False)
    wk8_d = nc.declare_dram_parameter("wk8", [128, H, NPR, 2, D], F8, isOutput=False)
    wr8_d = nc.declare_dram_parameter("wr8", [128, H, NPR, 2, D], F8, isOutput=False)
    wv8_d = nc.declare_dram_parameter("wv8", [128, NPR, 2, E], F8, isOutput=False)
    wp8_d = nc.declare_dram_parameter("wp8", [D, H // 2, 2, E], F8, isOutput=False)
    w18_d = nc.declare_dram_parameter("w18", [128, NCT, NPR, 2, 128], F8, isOutput=False)
    bqkr_d = nc.declare_dram_parameter("bqkr", [D, 3, H], F32, isOutput=False)
    bv_d = nc.declare_dram_parameter("bv", [1, E], F32, isOutput=False)
    bproj_d = nc.declare_dram_parameter("bproj", [1, E], F32, isOutput=False)
    ln1g_d = nc.declare_dram_parameter("ln1g", [1, E], F32, isOutput=False)
    ln1b_d = nc.declare_dram_parameter("ln1b", [1, E], F32, isOutput=False)
    bff1_d = nc.declare_dram_parameter("bff1", [128, NCT], F32, isOutput=False)
    w2_dt = F8 if ff2_fp8 else BF16
    w2_d = nc.declare_dram_parameter("w2", [128, NCT, E], w2_dt, isOutput=False)
    bff2_d = nc.declare_dram_parameter("bff2", [1, E], F32, isOutput=False)
    ln2g_d = nc.declare_dram_parameter("ln2g", [1, E], F32, isOutput=False)
    ln2b_d = nc.declare_dram_parameter("ln2b", [1, E], F32, isOutput=False)
    y_d = nc.declare_dram_parameter("y", [N, E], F32, isOutput=True)

    with tile.TileContext(nc) as tc, ExitStack() as ctx:
        # ---- whole-kernel pools ----
        persist = ctx.enter_context(tc.tile_pool(name="persist", bufs=1))
        wts = ctx.enter_context(tc.tile_pool(name="wts", bufs=1))
        act = ctx.enter_context(tc.tile_pool(name="act", bufs=1))

        ident = persist.tile([128, 128], F32)
        make_identity(nc, ident[:])
        eps_t = persist.tile([128, 1], F32)
        nc.vector.memset(eps_t[:], LN_EPS)
        expoff_t = persist.tile([128, 1], F32)
        nc.vector.memset(expoff_t[:], EXP_OFF)

        # ---- weight/x DMAs, transfer order = issue order: qkr(h0) operands
        # first so the tensor engine starts ~3us in ----
        xT8 = act.tile([128, NEC, N], F8, name="xT8")
        nc.sync.dma_start(out=xT8[:, :, 0:512], in_=xt8_d[:, :, 0:512])
        wq_sb = wts.tile([128, H, NPR, 2, D], F8, name="wq_sb")
        wk_sb = wts.tile([128, H, NPR, 2, D], F8, name="wk_sb")
        wr_sb = wts.tile([128, H, NPR, 2, D], F8, name="wr_sb")
        wv_sb = wts.tile([128, NPR, 2, E], F8, name="wv_sb")
        nc.sync.dma_start(out=wq_sb[:], in_=wq8_d[:])
        nc.sync.dma_start(out=wv_sb[:], in_=wv8_d[:])
        nc.sync.dma_start(out=wk_sb[:], in_=wk8_d[:])
        nc.sync.dma_start(out=wr_sb[:], in_=wr8_d[:])
        nc.sync.dma_start(out=xT8[:, :, 512:1024], in_=xt8_d[:, :, 512:1024])
        wp_sb = wts.tile([D, H // 2, 2, E], F8, name="wp_sb")
        nc.sync.dma_start(out=wp_sb[:], in_=wp8_d[:])
        # bulk weights last on the same queue: they transfer only after the
        # attention-critical items above, but well before ff1 needs them
        w1_sb = wts.tile([128, NCT, NPR, 2, 128], F8, name="w1_sb")
        nc.sync.dma_start(out=w1_sb[:], in_=w18_d[:])
        bff1_t = wts.tile([128, NCT], F32, name="bff1_t")
        nc.sync.dma_start(out=bff1_t[:], in_=bff1_d[:])
        bqkr_t = persist.tile([D, 3, H], F32)
        nc.sync.dma_start(out=bqkr_t[:], in_=bqkr_d[:])

        # warm-up matmuls (PE clock ramp)
        warm_t = persist.tile([128, 128], BF16)
        nc.vector.memset(warm_t[:], 0.0)
        with tc.tile_pool(name="warm_ps", bufs=2, space="PSUM") as warm_ps:
            for _ in range(12):
                wp_ = warm_ps.tile([128, 128], F32, tag="wp_", name="wp_")
                nc.tensor.matmul(wp_[:], warm_t[:], warm_t[:], start=True, stop=True)
                nc.tensor.matmul(wp_[:], warm_t[:], warm_t[:], start=True, stop=True)

        # ---- phases 1+2: v projection and attention share the qkr pools so
        # qkr(head 0) can be emitted before the v loop (fills the DMA wait) ----
        # last dim padded 97->128: dual-fp8 LDWEIGHTS requires full 32-aligned
        # chunk geometry; pad columns stay zero and emit into unused psum rows
        DP = 128
        v_aug = act.tile([128, H, NQT, DP], F8, name="v_aug")
        nc.vector.memset(v_aug[:, :, :, D : D + 1], C1)
        nc.vector.memset(v_aug[:, :, :, D + 1 : DP], 0.0)
        og8 = act.tile([D, H, N], F8, name="og8")  # gated out, scale 2^8
        with (
            tc.tile_pool(name="qkr", bufs=2) as qkr_pool,
            tc.tile_pool(name="expET", bufs=2) as exp_pool,
            tc.tile_pool(name="att_tmp", bufs=2) as tmp_pool,
            tc.tile_pool(name="qkr_ps", bufs=2, space="PSUM") as qkr_ps,
        ):
            def emit_qkr(h):
                dst = qkr_pool.tile([D, 3, N], BF16, tag="qkrT", name=f"qkrT{h}")
                for qh in range(2):
                    for si, w_sb in enumerate((wq_sb, wk_sb, wr_sb)):
                        qp = qkr_ps.tile([D, 512], F32, tag="qp", name="qp")
                        for p in range(NPR):
                            nc.tensor.matmul(
                                qp[:],
                                w_sb[:, h, p, :, :],
                                xT8[:, 2 * p : 2 * p + 2, qh * 512 : (qh + 1) * 512],
                                start=(p == 0),
                                stop=(p == NPR - 1),
                                perf_mode=DR,
                            )
                        dslc = dst[:, si, qh * 512 : (qh + 1) * 512]
                        if zero_bias:
                            nc.vector.tensor_copy(dslc, qp[:])
                        else:
                            nc.vector.tensor_scalar(
                                out=dslc, in0=qp[:],
                                scalar1=bqkr_t[:, si, h : h + 1], scalar2=None,
                                op0=ALU.add,
                            )
                return dst

            with tc.tile_pool(name="v_ps", bufs=2, space="PSUM") as v_ps:
                qkrT = emit_qkr(0)
                bv_bc = None
                if not zero_bias:
                    bv_bc = tmp_pool.tile([128, E], F32, tag="bv", name="bv_bc", bufs=1)
                    _bcast_dma(nc, bv_bc[:], bv_d[0:1, :])
                for qt in range(NQT):
                    vp = v_ps.tile([128, E], F32, tag="vp", name="vp")
                    for p in range(NPR):
                        for o, w in ESPL:
                            nc.tensor.matmul(
                                vp[:, o : o + w],
                                xT8[:, 2 * p : 2 * p + 2, qt * 128 : (qt + 1) * 128],
                                wv_sb[:, p, :, o : o + w],
                                start=(p == 0),
                                stop=(p == NPR - 1),
                                perf_mode=DR,
                            )
                    vph = vp.rearrange("k (h d) -> k h d", h=H)
                    if zero_bias:
                        nc.vector.tensor_scalar(
                            out=v_aug[:, :, qt, 0:D], in0=vph,
                            scalar1=V8_DESCALE, scalar2=None, op0=ALU.mult,
                        )
                    else:
                        nc.vector.scalar_tensor_tensor(
                            out=v_aug[:, :, qt, 0:D],
                            in0=vph, scalar=V8_DESCALE,
                            in1=bv_bc.rearrange("k (h d) -> k h d", h=H),
                            op0=ALU.mult, op1=ALU.add,
                        )
            with (
                tc.tile_pool(name="eng_ps", bufs=2, space="PSUM") as eng_ps,
                tc.tile_pool(name="att_ps", bufs=2, space="PSUM") as att_ps,
            ):
                for h in range(H):
                    cur = qkrT
                    expET = exp_pool.tile([128, NQT, N], F8, tag="expET", name=f"expET{h}")
                    for qh in range(2):
                        for g in range(4):
                            ep = eng_ps.tile([128, 2, 512], F32, tag="ep", name="ep")
                            for j in range(2):
                                kt = 2 * g + j
                                nc.tensor.matmul(
                                    ep[:, j, :],
                                    cur[:, 1, kt * 128 : (kt + 1) * 128],
                                    cur[:, 0, qh * 512 : (qh + 1) * 512],
                                    start=True,
                                    stop=True,
                                )
                            nc.scalar.activation(
                                out=expET[:, 2 * g : 2 * g + 2, qh * 512 : (qh + 1) * 512],
                                in_=ep[:],
                                func=AF.Exp,
                                scale=EXP_SCALE,
                                bias=expoff_t[:],
                            )
                    if h + 1 < H:
                        qkrT = emit_qkr(h + 1)
                    for qh in range(2):
                        op_ = att_ps.tile([128, 512], F32, tag="op", name="op")
                        for t2 in range(NQT // 2):
                            nc.tensor.matmul(
                                op_[:],
                                v_aug[:, h, 2 * t2 : 2 * t2 + 2, :],
                                expET[:, 2 * t2 : 2 * t2 + 2, qh * 512 : (qh + 1) * 512],
                                start=(t2 == 0),
                                stop=(t2 == NQT // 2 - 1),
                                perf_mode=DR,
                            )
                        sums = tmp_pool.tile([1, 512], F32, tag="sums", name="sums")
                        nc.vector.tensor_scalar(
                            out=sums[:], in0=op_[D : D + 1, :],
                            scalar1=SUMS_SCALE, scalar2=None, op0=ALU.mult,
                        )
                        recip = tmp_pool.tile([1, 512], F32, tag="recip", name="recip")
                        nc.vector.reciprocal_approx_fast(recip[:], sums[:])
                        bc = tmp_pool.tile([D, 512], F32, tag="bc", name="bc")
                        nc.gpsimd.partition_broadcast(bc[:], recip[:])
                        gated = tmp_pool.tile([D, 512], F32, tag="gated", name="gated")
                        nc.vector.tensor_tensor(
                            out=gated[:],
                            in0=op_[0:D, :],
                            in1=cur[:, 2, qh * 512 : (qh + 1) * 512],
                            op=ALU.mult,
                        )
                        nc.vector.tensor_tensor(
                            out=og8[:, h, qh * 512 : (qh + 1) * 512],
                            in0=gated[:],
                            in1=bc[:],
                            op=ALU.mult,
                        )

        # ---- phases 3+4: proj+LN1, then transposes/ff1/ff2 interleaved so
        # the LN1 tail hides under ff1(half0) and w2 is SBUF-resident ----
        x1_all = act.tile([128, NQT, E], F32, name="x1_all")
        x1T8 = act.tile([128, NEC, N], F8, name="x1T8")
        gT = act.tile([128, NCT, 512], BF16 if not ff2_fp8 else F8, name="gT")
        with (
            tc.tile_pool(name="xr", bufs=8) as xr_pool,
            tc.tile_pool(name="bcmid", bufs=1) as bcm_pool,
            tc.tile_pool(name="ln_tmp", bufs=2) as ln_pool,
            tc.tile_pool(name="w2res", bufs=1) as w2_pool,
            tc.tile_pool(name="ln2_tmp", bufs=2) as ln2_pool,
            tc.tile_pool(name="out", bufs=2) as out_pool,
        ):
            # residual rows must not queue behind the 4.5MB w2 transfer:
            # issue all of them first (transfers land during attention)
            xrs = []
            for qt in range(NQT):
                xr = xr_pool.tile([128, E], F32, tag="xr", name=f"xr{qt}")
                nc.sync.dma_start(out=xr[:], in_=x_d[qt * 128 : (qt + 1) * 128, :])
                xrs.append(xr)
            w2_sb = w2_pool.tile([128, NCT, E], w2_dt, name="w2_sb")
            nc.sync.dma_start(out=w2_sb[:], in_=w2_d[:])
            bcm = bce = None
            if not (identity_ln and zero_bias):
                bcm = bcm_pool.tile([128, 3, E], F32, tag="bcm", name="bcm")
                for i, d in enumerate((bproj_d, ln1g_d, ln1b_d)):
                    _bcast_dma(nc, bcm[:, i, :], d[0:1, :])
                bce = bcm_pool.tile([128, 3, E], F32, tag="bce", name="bce")
                for i, d in enumerate((bff2_d, ln2g_d, ln2b_d)):
                    _bcast_dma(nc, bce[:, i, :], d[0:1, :])

            with tc.tile_pool(name="y1_ps", bufs=2, space="PSUM") as y1_ps:
                for qt in range(NQT):
                    yp = y1_ps.tile([128, E], F32, tag="yp", name="yp")
                    for hp in range(H // 2):
                        for o, w in ESPL:
                            nc.tensor.matmul(
                                yp[:, o : o + w],
                                og8[:, 2 * hp : 2 * hp + 2, qt * 128 : (qt + 1) * 128],
                                wp_sb[:, hp, :, o : o + w],
                                start=(hp == 0),
                                stop=(hp == H // 2 - 1),
                                perf_mode=DR,
                            )
                    xr = xrs[qt]
                    if not zero_bias:
                        nc.vector.tensor_tensor(out=xr[:], in0=xr[:], in1=bcm[:, 0, :], op=ALU.add)
                    t1 = ln_pool.tile([128, E], F32, tag="lnt1", name="lnt1")
                    scr = ln_pool.tile([128, 8], F32, tag="lnscr", name="lnscr")
                    # guard against accumulate-into-garbage accum_out semantics
                    nc.vector.memset(scr[:, 0:2], 0.0)
                    nc.vector.scalar_tensor_tensor(
                        out=t1[:], in0=yp[:], scalar=PROJ_DESCALE, in1=xr[:],
                        op0=ALU.mult, op1=ALU.add, accum_out=scr[:, 0:1],
                    )
                    _ln_post(nc, ln_pool, t1[:], scr, x1_all[:, qt, :],
                             bcm[:, 1, :] if bcm is not None else None,
                             bcm[:, 2, :] if bcm is not None else None,
                             eps_t, identity_ln, on_scalar=(qt % 2 == 0))

            def emit_tp(tp_ps, qts, scalar_ecs):
                for qt in qts:
                    for ec in range(NEC):
                        pt1 = tp_ps.tile([128, 128], F32, tag="pt1", name="pt1")
                        nc.tensor.transpose(pt1[:], x1_all[:, qt, ec * 128 : (ec + 1) * 128], ident[:])
                        dst = x1T8[:, ec, qt * 128 : (qt + 1) * 128]
                        if ec in scalar_ecs:
                            # Copy lives in every act table: no table-load cost
                            nc.scalar.activation(out=dst, in_=pt1[:], func=AF.Copy, scale=SX)
                        else:
                            nc.vector.tensor_scalar(
                                out=dst, in0=pt1[:], scalar1=SX, scalar2=None, op0=ALU.mult,
                            )

            def emit_ff1(h1_ps, half):
                for g in range(NCT // 2):
                    hp_ = h1_ps.tile([128, 2, 512], F32, tag="h1", name="h1")
                    for j in range(2):
                        ct = 2 * g + j
                        for p in range(NPR):
                            nc.tensor.matmul(
                                hp_[:, j, :],
                                w1_sb[:, ct, p, :, :],
                                x1T8[:, 2 * p : 2 * p + 2, half * 512 : (half + 1) * 512],
                                start=(p == 0),
                                stop=(p == NPR - 1),
                                perf_mode=DR,
                            )
                    if zero_bias:
                        nc.scalar.activation(
                            out=gT[:, 2 * g : 2 * g + 2, :], in_=hp_[:],
                            func=AF.Gelu, scale=FF1_DESCALE,
                        )
                    else:
                        for j in range(2):
                            ct = 2 * g + j
                            nc.scalar.activation(
                                out=gT[:, ct, :], in_=hp_[:, j, :],
                                func=AF.Gelu,
                                bias=bff1_t[:, ct : ct + 1], scale=FF1_DESCALE,
                            )

            def emit_ff2_pass(y2_ps, half, iqs, ln_on_scalar):
                y2p = {iq: y2_ps.tile([128, E], F32, tag=f"y2_{iq % 2}", name=f"y2_{half}_{iq}")
                       for iq in iqs}
                for ct in range(NCT):
                    for iq in iqs:
                        for o, w in ESPL:
                            nc.tensor.matmul(
                                y2p[iq][:, o : o + w],
                                gT[:, ct, iq * 128 : (iq + 1) * 128],
                                w2_sb[:, ct, o : o + w],
                                start=(ct == 0),
                                stop=(ct == NCT - 1),
                            )
                scrs = {}
                for iq in iqs:
                    qt = half * 4 + iq
                    x1q = x1_all[:, qt, :]
                    if not zero_bias:
                        nc.vector.tensor_tensor(out=x1q, in0=x1q, in1=bce[:, 0, :], op=ALU.add)
                    scr = ln2_pool.tile([128, 8], F32, tag="ln2scr", name="ln2scr", bufs=4)
                    scrs[iq] = scr
                    nc.vector.memset(scr[:, 0:2], 0.0)
                    nc.vector.scalar_tensor_tensor(
                        out=x1q, in0=y2p[iq][:],
                        scalar=FF2_DESCALE if ff2_fp8 else 1.0, in1=x1q,
                        op0=ALU.mult, op1=ALU.add, accum_out=scr[:, 0:1],
                    )
                for iq in iqs:
                    qt = half * 4 + iq
                    yout = out_pool.tile([128, E], F32, tag="yout", name="yout")
                    _ln_post(nc, ln2_pool, x1_all[:, qt, :], scrs[iq], yout[:],
                             bce[:, 1, :] if bce is not None else None,
                             bce[:, 2, :] if bce is not None else None,
                             eps_t, identity_ln, on_scalar=ln_on_scalar)
                    nc.sync.dma_start(out=y_d[qt * 128 : (qt + 1) * 128, :], in_=yout[:])

            with (
                tc.tile_pool(name="tp1_ps", bufs=2, space="PSUM") as tp1_ps,
                tc.tile_pool(name="h1a_ps", bufs=3, space="PSUM") as h1a_ps,
            ):
                emit_tp(tp1_ps, range(0, 4), scalar_ecs=())
                emit_ff1(h1a_ps, 0)
            with (
                tc.tile_pool(name="y2a_ps", bufs=1, space="PSUM") as y2a_ps,
                tc.tile_pool(name="tp2_ps", bufs=2, space="PSUM") as tp2_ps,
            ):
                emit_tp(tp2_ps, range(4, 8), scalar_ecs=(0, 2, 4))
                # half0's LN2s execute while ff1(half1) hogs scalar with gelus
                emit_ff2_pass(y2a_ps, 0, (0, 1), ln_on_scalar=False)
                emit_ff2_pass(y2a_ps, 0, (2, 3), ln_on_scalar=False)
            with tc.tile_pool(name="h1b_ps", bufs=3, space="PSUM") as h1b_ps:
                emit_ff1(h1b_ps, 1)
            with tc.tile_pool(name="y2b_ps", bufs=1, space="PSUM") as y2b_ps:
                # single-iq passes: each LN2+DMA chain hides under the next
                # pass's matmuls, leaving only one chain after the last one
                emit_ff2_pass(y2b_ps, 1, (0,), ln_on_scalar=True)
                emit_ff2_pass(y2b_ps, 1, (1,), ln_on_scalar=False)
                emit_ff2_pass(y2b_ps, 1, (2,), ln_on_scalar=True)
                emit_ff2_pass(y2b_ps, 1, (3,), ln_on_scalar=False)

    nc.compile()
    return nc


_NC_CACHE = {}
FF2_FP8 = False


def _get_nc(identity_ln=False, zero_bias=False):
    key = (identity_ln, zero_bias, FF2_FP8)
    if key not in _NC_CACHE:
        _NC_CACHE[key] = _build(identity_ln, zero_bias, FF2_FP8)
    return _NC_CACHE[key]


def _q8(a, scale):
    return (np.asarray(a, np.float32) * scale).astype(ml_dtypes.float8_e4m3)


def _prep_weights(w_qkvr, b_qkvr, w_proj, b_proj, ln1_g, ln1_b,
                  w_ff1, b_ff1, w_ff2, b_ff2, ln2_g, ln2_b):
    w4 = np.asarray(w_qkvr, np.float32).reshape(E, H, D, 4)
    b4 = np.asarray(b_qkvr, np.float32).reshape(H, D, 4)
    s = np.float32(1.0 / np.sqrt(E))

    def head_pairs(w):  # [E, (h d)] -> [128, h, pair, 2, d], fp8*SW
        return np.ascontiguousarray(
            _q8(w, SW).reshape(NPR, 2, 128, H, D).transpose(2, 3, 0, 1, 4)
        )

    wq8 = head_pairs(w4[..., 0].reshape(E, E))
    wk8 = head_pairs(w4[..., 1].reshape(E, E))
    wr8 = head_pairs(w4[..., 3].reshape(E, E))
    wv8 = np.ascontiguousarray(
        _q8(w4[..., 2].reshape(E, E) * s, SWV).reshape(NPR, 2, 128, E).transpose(2, 0, 1, 3)
    )
    wp8 = np.ascontiguousarray(
        _q8(w_proj, SW).reshape(H // 2, 2, D, E).transpose(2, 0, 1, 3)
    )
    w18 = np.ascontiguousarray(
        _q8(w_ff1, SW).reshape(NPR, 2, 128, NCT, 128).transpose(2, 3, 0, 1, 4)
    )
    # raw-scale biases: q/k/r carry 2^13, v carries 2^17
    bqkr = np.ascontiguousarray(
        np.stack([b4[..., 0], b4[..., 1], b4[..., 3]], 0).transpose(2, 0, 1)
    ) * np.float32(SX * SW)
    bv = np.ascontiguousarray((b4[..., 2] * s).reshape(1, E)) * np.float32(SX * SWV * V8_DESCALE)
    bff1 = np.ascontiguousarray(np.asarray(b_ff1, np.float32).reshape(NCT, 128).T)
    if FF2_FP8:
        w2 = _q8(w_ff2, SWV)
    else:
        w2 = np.asarray(w_ff2, np.float32).astype(ml_dtypes.bfloat16)
    w2 = np.ascontiguousarray(w2.reshape(NCT, 128, E).transpose(1, 0, 2))
    return {
        "wq8": wq8, "wk8": wk8, "wr8": wr8, "wv8": wv8, "wp8": wp8, "w18": w18,
        "bqkr": bqkr, "bv": bv,
        "bproj": np.asarray(b_proj, np.float32).reshape(1, E).copy(),
        "ln1g": np.asarray(ln1_g, np.float32).reshape(1, E).copy(),
        "ln1b": np.asarray(ln1_b, np.float32).reshape(1, E).copy(),
        "bff1": bff1, "w2": w2,
        "bff2": np.asarray(b_ff2, np.float32).reshape(1, E).copy(),
        "ln2g": np.asarray(ln2_g, np.float32).reshape(1, E).copy(),
        "ln2b": np.asarray(ln2_b, np.float32).reshape(1, E).copy(),
    }


def _in_maps(inputs):
    x = np.asarray(inputs["x"], np.float32)
    shared = _prep_weights(
        inputs["w_qkvr"], inputs["b_qkvr"], inputs["w_proj"], inputs["b_proj"],
        inputs["ln1_g"], inputs["ln1_b"], inputs["w_ff1"], inputs["b_ff1"],
        inputs["w_ff2"], inputs["b_ff2"], inputs["ln2_g"], inputs["ln2_b"],
    )
    maps = []
    for i in range(N_CORES):
        xi = np.ascontiguousarray(x[i])
        xt8 = np.ascontiguousarray(
            _q8(xi.T, SX).reshape(NEC, 128, N).transpose(1, 0, 2)
        )
        maps.append({**shared, "x": xi, "xt8": xt8})
    return maps


def _flags(inputs):
    z = lambda k: not np.any(np.asarray(inputs[k]))
    one = lambda k: bool(np.all(np.asarray(inputs[k]) == 1.0))
    identity_ln = (one("ln1_g") and z("ln1_b") and one("ln2_g") and z("ln2_b"))
    zero_bias = (z("b_qkvr") and z("b_proj") and z("b_ff2") and z("b_ff1"))
    return identity_ln, zero_bias


def kernel(**inputs) -> np.ndarray:
    identity_ln, zero_bias = _flags(inputs)
    nc = _get_nc(identity_ln, zero_bias)
    res = run_bass_kernel_spmd(nc, _in_maps(inputs), core_ids=list(range(N_CORES)))
    return np.stack([res.results[i]["y"] for i in range(N_CORES)], axis=0)
